# revision 24
# baseline (speedup 1.0000x reference)
import sys

sys.path.insert(0, "/opt/trn_rl_repo")

import numpy as np

G, E, N, H = 8, 8192, 512, 32
NP1 = N + 1          # 513
T = N * N            # 262144 tokens per graph
NG = 512             # token groups of 512
NQ = NG // 4         # 128 quads
LAG = 8              # stage1 -> stage2 lag in groups (multiple of 4)
RL = LAG // 2 + 4    # hh ring size in pairs


# ----------------------------------------------------------------- device code
def build(nc, outs, ins):
    from contextlib import ExitStack

    import concourse.tile as tile
    from concourse import mybir

    f32 = mybir.dt.float32
    fp16 = mybir.dt.float16
    Relu = mybir.ActivationFunctionType.Relu
    Alu = mybir.AluOpType

    out2 = outs["out"]               # [128, T//4] f32; row = (g%4)*32+h
    xln = ins["xln"]                 # [112, T] fp16 (X rows 0:56, xi rows 56:112)
    w1a = ins["w1a"]                 # [112, 64] fp16 = [A1; A1]
    w1b = ins["w1b"]                 # [112, 64] fp16 = [a1; a1]
    w2a = ins["w2a"]                 # [128, 64] fp16 = [[A2;0] | [0;A2]]
    w2b = ins["w2b"]                 # [128, 64] fp16 = [[a2;0] | [0;a2]]

    with tile.TileContext(nc) as tc, ExitStack() as ctx:
        cst = ctx.enter_context(tc.tile_pool(name="cst", bufs=1))
        w1a_s = cst.tile([112, 64], fp16)
        nc.sync.dma_start(out=w1a_s[:], in_=w1a[:])
        w1b_s = cst.tile([112, 64], fp16)
        nc.sync.dma_start(out=w1b_s[:], in_=w1b[:])
        w2a_s = cst.tile([128, 64], fp16)
        nc.sync.dma_start(out=w2a_s[:], in_=w2a[:])
        w2b_s = cst.tile([128, 64], fp16)
        nc.sync.dma_start(out=w2b_s[:], in_=w2b[:])

        # x input staged per hex (16 groups) via SWDGE striping
        xin = ctx.enter_context(tc.tile_pool(name="xin", bufs=3))
        xo_tiles = {}

        def load_hex(o):
            if o >= NQ // 4 or o in xo_tiles:
                return
            t = xin.tile([112, 8192], fp16, tag="xo")
            nc.gpsimd.dma_start(out=t[:], in_=xln[:, o * 8192:(o + 1) * 8192])
            xo_tiles[o] = t

        for o in range(2):
            load_hex(o)

        hhp = ctx.enter_context(tc.tile_pool(name="hhp", bufs=RL))
        etp = ctx.enter_context(tc.tile_pool(name="etp", bufs=RL))
        osp = ctx.enter_context(tc.tile_pool(name="osp", bufs=3))
        ps1 = ctx.enter_context(tc.tile_pool(name="ps1", bufs=4, space="PSUM"))
        ps2 = ctx.enter_context(tc.tile_pool(name="ps2", bufs=3, space="PSUM"))

        ht_ring = [None] * RL            # H pair tiles [128, 512]
        et_ring = [None] * RL            # eta pair tiles [128, 512]
        hps_ring = [None] * 4

        def eta_op(P):
            # eta(P) = fp16(relu(h) - H); lags a pair so it runs on DVE
            # concurrently with the next pair's H-op on Act
            nc.vector.scalar_tensor_tensor(
                out=et_ring[P % RL][:], in0=hps_ring[P % 4][:],
                scalar=0.0, in1=ht_ring[P % RL][:],
                op0=Alu.max, op1=Alu.subtract,
            )

        NB = NG // 4          # 128 blocks of 4 groups (2 pairs)
        BLAG = LAG // 4       # lag in blocks

        def stage1_pair(P):
            # pair-packed psum [128, 512]: rows 0:64 = h(2P), 64:128 = h(2P+1)
            # 8 matmuls on 4 disjoint 32-wide PE column tiles -> concurrent
            hps = ps1.tile([128, 512], f32, tag="hps")
            hps_ring[P % 4] = hps
            rhs = []
            for pj in (0, 1):
                g = 2 * P + pj
                o, j16 = g // 16, g % 16
                if j16 == 0:
                    load_hex(o + 2)
                xo = xo_tiles[o]
                rhs.append(xo[0:112, j16 * 512:(j16 + 1) * 512])
            for c in range(4):
                q0 = 32 * c
                nc.tensor.matmul(out=hps[q0:q0 + 32, :],
                                 lhsT=w1a_s[:, (c % 2) * 32:(c % 2) * 32 + 32],
                                 rhs=rhs[c // 2],
                                 start=True, stop=False,
                                 tile_position=(0, q0),
                                 skip_group_check=True)
            for c in range(4):
                q0 = 32 * c
                nc.tensor.matmul(out=hps[q0:q0 + 32, :],
                                 lhsT=w1b_s[:, (c % 2) * 32:(c % 2) * 32 + 32],
                                 rhs=rhs[c // 2],
                                 start=False, stop=True,
                                 tile_position=(0, q0),
                                 skip_group_check=True)

        def h_op(P):
            ht = hhp.tile([128, 512], fp16, tag="ht")
            et = etp.tile([128, 512], fp16, tag="et")
            nc.scalar.activation(out=ht[:], in_=hps_ring[P % 4][:], func=Relu)
            ht_ring[P % RL] = ht
            et_ring[P % RL] = et

        for B in range(NB + BLAG):
            if B < NB:
                # ---- stage1: 2 pairs, col-tiled concurrent matmul pairs
                stage1_pair(2 * B)
                stage1_pair(2 * B + 1)
                h_op(2 * B)
                h_op(2 * B + 1)
                if B >= 1:
                    eta_op(2 * B - 1)
                eta_op(2 * B)
                if B == NB - 1:
                    eta_op(2 * B + 1)
            if B >= BLAG:
                # ---- stage2: 16 K=64 matmuls on 4 disjoint PE tiles
                B2 = B - BLAG
                ops = ps2.tile([128, 512], f32, tag="ops")
                for j4 in range(4):
                    g2 = 4 * B2 + j4
                    P2, pj2 = g2 // 2, g2 % 2
                    w0 = 32 * pj2     # weight col-block selects even/odd rows
                    ht2 = ht_ring[P2 % RL]
                    et2 = et_ring[P2 % RL]
                    q0 = j4 * 32
                    tp = (0, q0)
                    dst = ops[q0:q0 + 32, :]
                    nc.tensor.matmul(out=dst, lhsT=w2a_s[:, w0:w0 + 32],
                                     rhs=ht2[:], start=True, stop=False,
                                     tile_position=tp)
                    nc.tensor.matmul(out=dst, lhsT=w2a_s[:, w0:w0 + 32],
                                     rhs=et2[:], start=False, stop=False,
                                     tile_position=tp)
                    nc.tensor.matmul(out=dst, lhsT=w2b_s[:, w0:w0 + 32],
                                     rhs=ht2[:], start=False, stop=False,
                                     tile_position=tp)
                    nc.tensor.matmul(out=dst, lhsT=w2b_s[:, w0:w0 + 32],
                                     rhs=et2[:], start=False, stop=True,
                                     tile_position=tp)
                # stage output in [128, 2048] tiles; one DMA per four blocks
                if B2 % 4 == 0:
                    osb = osp.tile([128, 2048], f32, tag="osb")
                quarter = osb[:, (B2 % 4) * 512:(B2 % 4) * 512 + 512]
                if B2 % 3 == 2:
                    nc.vector.tensor_copy(out=quarter, in_=ops[:])
                else:
                    nc.scalar.copy(out=quarter, in_=ops[:])
                if B2 % 4 == 3:
                    nc.sync.dma_start(
                        out=out2[:, (B2 - 3) * 512:(B2 + 1) * 512], in_=osb[:]
                    )


# ----------------------------------------------------------------- host prep
def _split16(x):
    hi = x.astype(np.float16)
    lo = (x - hi.astype(np.float32)).astype(np.float16)
    return hi, lo


def prep_weights(inputs):
    w1 = np.zeros((56, 64), np.float32)
    w1[0:28, 0:32] = np.asarray(inputs["ang_w1"], np.float32)
    w1[28:56, 32:64] = np.asarray(inputs["md_w1"], np.float32)
    b1 = np.concatenate([np.asarray(inputs["ang_b1"]),
                         np.asarray(inputs["md_b1"])]).astype(np.float32)
    assert not np.any(b1), "kernel assumes zero hidden bias"
    w2 = np.concatenate([np.asarray(inputs["ang_w2"], np.float32),
                         np.asarray(inputs["md_w2"], np.float32)], 0)
    A1, a1 = _split16(w1)
    A2, a2 = _split16(w2)
    w1a = np.concatenate([A1, A1], 0)                       # [112, 64]
    w1b = np.concatenate([a1, a1], 0)                       # [112, 64]
    z = np.zeros((64, 32), np.float16)
    w2a = np.block([[A2, z], [z, A2]])                      # [128, 64]
    w2b = np.block([[a2, z], [z, a2]])                      # [128, 64]
    b2 = (np.asarray(inputs["ang_b2"]) + np.asarray(inputs["md_b2"]))
    return w1a, w1b, w2a, w2b, b2.astype(np.float32)


def prep_x(g, inputs):
    ang = np.asarray(inputs["angle"][g], np.float32).reshape(T, 28)
    dst = np.asarray(inputs["dists"][g], np.float32).reshape(T, 28)
    x = np.concatenate([ang, dst], 1).T                     # [56, T]
    X, xi = _split16(np.ascontiguousarray(x))
    return np.concatenate([X, xi], 0)                       # [112, T]


def edge_bias_host(g, inputs, full):
    """Exact f32 edge-embedding scatter, matching the reference."""
    ef = np.asarray(inputs["edge_feat"][g], np.float32)
    ei = np.asarray(inputs["edge_index"][g]).astype(np.int64)
    mask = np.asarray(inputs["edge_mask"][g]).astype(bool)
    nlig = max(int(inputs["num_ligand_atoms"][g]), 1)

    t0 = ef[:, 0].astype(np.int32)
    t1 = ef[:, 1].astype(np.int32)
    t2 = ef[:, 2].astype(np.int32)
    d = ef[:, 3:4]                                          # [E,1]
    src, tgt = ei[0], ei[1]
    src_l = (src > 0) & (src < nlig)
    tgt_l = (tgt > 0) & (tgt < nlig)

    dw1 = np.asarray(inputs["dist_w1"], np.float32)
    db1 = np.asarray(inputs["dist_b1"], np.float32)
    dw2 = np.asarray(inputs["dist_w2"], np.float32)
    db2 = np.asarray(inputs["dist_b2"], np.float32)
    demb = np.maximum(d @ dw1 + db1, 0.0) @ dw2 + db2       # [E, 32]

    sidx = np.clip(t0 * 4 + t1 * 2 + t2, 0, 19)
    structural = np.asarray(inputs["struct_emb"], np.float32)[sidx]
    pidx = np.clip(t1, 0, 14)
    both_l = src_l & tgt_l
    both_p = (~src_l) & (~tgt_l)
    plip = np.where(
        both_l[:, None], np.asarray(inputs["plip_lig"], np.float32)[pidx],
        np.where(both_p[:, None], np.asarray(inputs["plip_prot"], np.float32)[pidx],
                 np.asarray(inputs["plip_inter"], np.float32)[pidx]))
    emb = np.where((t0 <= 1)[:, None], structural,
                   np.where((t0 == 5)[:, None], plip, 0.0)) + demb
    emb = emb * mask[:, None].astype(np.float32)

    flat = full.reshape(-1)
    cell = ((src + 1) * NP1 + (tgt + 1)).astype(np.int64)   # [E]
    idx = (np.arange(H, dtype=np.int64) * (NP1 * NP1))[None, :] + cell[:, None]
    np.add.at(flat, idx.ravel(), emb.astype(np.float32).ravel())


_IN_SPECS = [
    ("xln", (112, T), "float16"),
    ("w1a", (112, 64), "float16"),
    ("w1b", (112, 64), "float16"),
    ("w2a", (128, 64), "float16"),
    ("w2b", (128, 64), "float16"),
]


def _build_nc():
    from concourse import bacc, mybir

    nc = bacc.Bacc(
        "TRN2",
        target_bir_lowering=False,
        debug=False,
        enable_asserts=False,
        num_devices=8,
    )
    ins = {}
    for name, shape, dt in _IN_SPECS:
        h = nc.dram_tensor(name, list(shape), getattr(mybir.dt, dt),
                           kind="ExternalInput")
        ins[name] = h[:]
    out_h = nc.dram_tensor("out", [128, T // 4], mybir.dt.float32,
                           kind="ExternalOutput")
    build(nc, {"out": out_h[:]}, ins)
    nc.compile()
    return nc


def kernel(_trace=False, **inputs):
    from concourse.bass_utils import run_bass_kernel_spmd

    w1a, w1b, w2a, w2b, b2 = prep_weights(inputs)
    in_maps = []
    for g in range(G):
        in_maps.append(dict(xln=prep_x(g, inputs), w1a=w1a, w1b=w1b,
                            w2a=w2a, w2b=w2b))

    nc = _build_nc()
    res = run_bass_kernel_spmd(nc, in_maps, core_ids=list(range(G)),
                               trace=_trace)
    if _trace:
        print("HW exec time:", res.exec_time_ns, "ns  (mean:",
              res.mean_exec_time_ns, "ns, slowest core:",
              res.max_exec_time_core_id, ")")
        if res.instructions_and_trace:
            print("trace:", res.instructions_and_trace[1])

    attn = np.asarray(inputs["attn_bias"], np.float32)      # [G, 513, 513]
    virt = np.asarray(inputs["virt"], np.float32).reshape(H)
    outs = []
    for g, r in enumerate(res.results):
        # device rows: (g%4)*32 + h; cols: (g//4)*512 + c
        dev = r["out"].reshape(4, 32, NQ, 512).transpose(1, 2, 0, 3)
        dev = dev.reshape(32, T).reshape(32, N, N)          # [H, i, j]
        full = np.empty((H, NP1, NP1), np.float32)
        full[:, 1:, 1:] = dev + b2[:, None, None] + attn[g][None, 1:, 1:]
        full[:, 1:, 0] = attn[g][None, 1:, 0] + virt[:, None]
        full[:, 0, :] = attn[g][None, 0, :] + virt[:, None]
        edge_bias_host(g, inputs, full)
        outs.append(full)
    return np.stack(outs).astype(np.float32)


# revision 28
# speedup vs baseline: 1.1514x; 1.1514x over previous
import sys

sys.path.insert(0, "/opt/trn_rl_repo")

import numpy as np

G, E, N, H = 8, 8192, 512, 32
NP1 = N + 1          # 513
T = N * N            # 262144 tokens per graph
NG = 512             # token groups of 512
NQ = NG // 4         # 128 quads
LAG = 8              # stage1 -> stage2 lag in groups (multiple of 4)
RL = LAG // 2 + 4    # hh ring size in pairs


# ----------------------------------------------------------------- device code
def build(nc, outs, ins):
    from contextlib import ExitStack

    import concourse.tile as tile
    from concourse import mybir

    f32 = mybir.dt.float32
    fp16 = mybir.dt.float16
    Relu = mybir.ActivationFunctionType.Relu
    Alu = mybir.AluOpType

    out2 = outs["out"]               # [128, T//4] f32; row = (g%4)*32+h
    xln = ins["xln"]                 # [112, T] fp16 (X rows 0:56, xi rows 56:112)
    w1a = ins["w1a"]                 # [112, 64] fp16 = [A1; A1]
    w1b = ins["w1b"]                 # [112, 64] fp16 = [a1; a1]
    w2a = ins["w2a"]                 # [128, 64] fp16 = [[A2;0] | [0;A2]]
    w2b = ins["w2b"]                 # [128, 64] fp16 = [[a2;0] | [0;a2]]

    with tile.TileContext(nc) as tc, ExitStack() as ctx:
        cst = ctx.enter_context(tc.tile_pool(name="cst", bufs=1))
        w1a_s = cst.tile([112, 64], fp16)
        nc.sync.dma_start(out=w1a_s[:], in_=w1a[:])
        w1b_s = cst.tile([112, 64], fp16)
        nc.sync.dma_start(out=w1b_s[:], in_=w1b[:])
        w2a_s = cst.tile([128, 64], fp16)
        nc.sync.dma_start(out=w2a_s[:], in_=w2a[:])
        w2b_s = cst.tile([128, 64], fp16)
        nc.sync.dma_start(out=w2b_s[:], in_=w2b[:])

        # x input staged per oct (8 groups) via SWDGE striping
        xin = ctx.enter_context(tc.tile_pool(name="xin", bufs=4))
        xo_tiles = {}

        def load_oct(o):
            if o >= NQ // 2 or o in xo_tiles:
                return
            t = xin.tile([112, 4096], fp16, tag="xo")
            nc.gpsimd.dma_start(out=t[:], in_=xln[:, o * 4096:(o + 1) * 4096])
            xo_tiles[o] = t

        for o in range(2):
            load_oct(o)

        hhp = ctx.enter_context(tc.tile_pool(name="hhp", bufs=RL))
        etp = ctx.enter_context(tc.tile_pool(name="etp", bufs=RL))
        osp = ctx.enter_context(tc.tile_pool(name="osp", bufs=3))
        ps1 = ctx.enter_context(tc.tile_pool(name="ps1", bufs=4, space="PSUM"))
        ps2 = ctx.enter_context(tc.tile_pool(name="ps2", bufs=3, space="PSUM"))

        ht_ring = [None] * RL            # H pair tiles [128, 512]
        et_ring = [None] * RL            # eta pair tiles [128, 512]
        hps_ring = [None] * 4

        def eta_op(P):
            # eta(P) = fp16(relu(h) - H); lags a pair so it runs on DVE
            # concurrently with the next pair's H-op on Act
            nc.vector.scalar_tensor_tensor(
                out=et_ring[P % RL][:], in0=hps_ring[P % 4][:],
                scalar=0.0, in1=ht_ring[P % RL][:],
                op0=Alu.max, op1=Alu.subtract,
            )

        NB = NG // 4          # 128 blocks of 4 groups (2 pairs)
        BLAG = LAG // 4       # lag in blocks

        def stage1_pair(P):
            # pair-packed psum [128, 512]: rows 0:64 = h(2P), 64:128 = h(2P+1)
            # 8 matmuls on 4 disjoint 32-wide PE column tiles -> concurrent
            hps = ps1.tile([128, 512], f32, tag="hps")
            hps_ring[P % 4] = hps
            rhs = []
            for pj in (0, 1):
                g = 2 * P + pj
                o, j8 = g // 8, g % 8
                if j8 == 0:
                    load_oct(o + 2)
                xo = xo_tiles[o]
                rhs.append(xo[0:112, j8 * 512:(j8 + 1) * 512])
            for c in range(4):
                q0 = 32 * c
                nc.tensor.matmul(out=hps[q0:q0 + 32, :],
                                 lhsT=w1a_s[:, (c % 2) * 32:(c % 2) * 32 + 32],
                                 rhs=rhs[c // 2],
                                 start=True, stop=False,
                                 tile_position=(0, q0),
                                 skip_group_check=True)
            for c in range(4):
                q0 = 32 * c
                nc.tensor.matmul(out=hps[q0:q0 + 32, :],
                                 lhsT=w1b_s[:, (c % 2) * 32:(c % 2) * 32 + 32],
                                 rhs=rhs[c // 2],
                                 start=False, stop=True,
                                 tile_position=(0, q0),
                                 skip_group_check=True)

        def h_op(P):
            ht = hhp.tile([128, 512], fp16, tag="ht")
            et = etp.tile([128, 512], fp16, tag="et")
            nc.scalar.activation(out=ht[:], in_=hps_ring[P % 4][:], func=Relu)
            ht_ring[P % RL] = ht
            et_ring[P % RL] = et

        for B in range(NB + BLAG):
            if B < NB:
                # ---- stage1: 2 pairs, col-tiled concurrent matmul pairs
                stage1_pair(2 * B)
                stage1_pair(2 * B + 1)
                h_op(2 * B)
                h_op(2 * B + 1)
                if B >= 1:
                    eta_op(2 * B - 1)
                eta_op(2 * B)
                if B == NB - 1:
                    eta_op(2 * B + 1)
            if B >= BLAG:
                # ---- stage2: 16 K=64 matmuls on 4 disjoint PE tiles
                B2 = B - BLAG
                ops = ps2.tile([128, 512], f32, tag="ops")
                for j4 in range(4):
                    g2 = 4 * B2 + j4
                    P2, pj2 = g2 // 2, g2 % 2
                    w0 = 32 * pj2     # weight col-block selects even/odd rows
                    ht2 = ht_ring[P2 % RL]
                    et2 = et_ring[P2 % RL]
                    q0 = j4 * 32
                    tp = (0, q0)
                    dst = ops[q0:q0 + 32, :]
                    nc.tensor.matmul(out=dst, lhsT=w2a_s[:, w0:w0 + 32],
                                     rhs=ht2[:], start=True, stop=False,
                                     tile_position=tp)
                    nc.tensor.matmul(out=dst, lhsT=w2a_s[:, w0:w0 + 32],
                                     rhs=et2[:], start=False, stop=False,
                                     tile_position=tp)
                    nc.tensor.matmul(out=dst, lhsT=w2b_s[:, w0:w0 + 32],
                                     rhs=ht2[:], start=False, stop=False,
                                     tile_position=tp)
                    nc.tensor.matmul(out=dst, lhsT=w2b_s[:, w0:w0 + 32],
                                     rhs=et2[:], start=False, stop=True,
                                     tile_position=tp)
                # stage output in [128, 1024] tiles; one DMA per two blocks
                if B2 % 2 == 0:
                    osb = osp.tile([128, 1024], f32, tag="osb")
                half = osb[:, (B2 % 2) * 512:(B2 % 2) * 512 + 512]
                if B2 % 3 == 2:
                    nc.vector.tensor_copy(out=half, in_=ops[:])
                else:
                    nc.scalar.copy(out=half, in_=ops[:])
                if B2 % 2 == 1:
                    dma_eng = nc.sync if B2 % 4 == 1 else nc.scalar
                    dma_eng.dma_start(
                        out=out2[:, (B2 - 1) * 512:(B2 + 1) * 512], in_=osb[:]
                    )


# ----------------------------------------------------------------- host prep
def _split16(x):
    hi = x.astype(np.float16)
    lo = (x - hi.astype(np.float32)).astype(np.float16)
    return hi, lo


def prep_weights(inputs):
    w1 = np.zeros((56, 64), np.float32)
    w1[0:28, 0:32] = np.asarray(inputs["ang_w1"], np.float32)
    w1[28:56, 32:64] = np.asarray(inputs["md_w1"], np.float32)
    b1 = np.concatenate([np.asarray(inputs["ang_b1"]),
                         np.asarray(inputs["md_b1"])]).astype(np.float32)
    assert not np.any(b1), "kernel assumes zero hidden bias"
    w2 = np.concatenate([np.asarray(inputs["ang_w2"], np.float32),
                         np.asarray(inputs["md_w2"], np.float32)], 0)
    A1, a1 = _split16(w1)
    A2, a2 = _split16(w2)
    w1a = np.concatenate([A1, A1], 0)                       # [112, 64]
    w1b = np.concatenate([a1, a1], 0)                       # [112, 64]
    z = np.zeros((64, 32), np.float16)
    w2a = np.block([[A2, z], [z, A2]])                      # [128, 64]
    w2b = np.block([[a2, z], [z, a2]])                      # [128, 64]
    b2 = (np.asarray(inputs["ang_b2"]) + np.asarray(inputs["md_b2"]))
    return w1a, w1b, w2a, w2b, b2.astype(np.float32)


def prep_x(g, inputs):
    ang = np.asarray(inputs["angle"][g], np.float32).reshape(T, 28)
    dst = np.asarray(inputs["dists"][g], np.float32).reshape(T, 28)
    x = np.concatenate([ang, dst], 1).T                     # [56, T]
    X, xi = _split16(np.ascontiguousarray(x))
    return np.concatenate([X, xi], 0)                       # [112, T]


def edge_bias_host(g, inputs, full):
    """Exact f32 edge-embedding scatter, matching the reference."""
    ef = np.asarray(inputs["edge_feat"][g], np.float32)
    ei = np.asarray(inputs["edge_index"][g]).astype(np.int64)
    mask = np.asarray(inputs["edge_mask"][g]).astype(bool)
    nlig = max(int(inputs["num_ligand_atoms"][g]), 1)

    t0 = ef[:, 0].astype(np.int32)
    t1 = ef[:, 1].astype(np.int32)
    t2 = ef[:, 2].astype(np.int32)
    d = ef[:, 3:4]                                          # [E,1]
    src, tgt = ei[0], ei[1]
    src_l = (src > 0) & (src < nlig)
    tgt_l = (tgt > 0) & (tgt < nlig)

    dw1 = np.asarray(inputs["dist_w1"], np.float32)
    db1 = np.asarray(inputs["dist_b1"], np.float32)
    dw2 = np.asarray(inputs["dist_w2"], np.float32)
    db2 = np.asarray(inputs["dist_b2"], np.float32)
    demb = np.maximum(d @ dw1 + db1, 0.0) @ dw2 + db2       # [E, 32]

    sidx = np.clip(t0 * 4 + t1 * 2 + t2, 0, 19)
    structural = np.asarray(inputs["struct_emb"], np.float32)[sidx]
    pidx = np.clip(t1, 0, 14)
    both_l = src_l & tgt_l
    both_p = (~src_l) & (~tgt_l)
    plip = np.where(
        both_l[:, None], np.asarray(inputs["plip_lig"], np.float32)[pidx],
        np.where(both_p[:, None], np.asarray(inputs["plip_prot"], np.float32)[pidx],
                 np.asarray(inputs["plip_inter"], np.float32)[pidx]))
    emb = np.where((t0 <= 1)[:, None], structural,
                   np.where((t0 == 5)[:, None], plip, 0.0)) + demb
    emb = emb * mask[:, None].astype(np.float32)

    flat = full.reshape(-1)
    cell = ((src + 1) * NP1 + (tgt + 1)).astype(np.int64)   # [E]
    idx = (np.arange(H, dtype=np.int64) * (NP1 * NP1))[None, :] + cell[:, None]
    np.add.at(flat, idx.ravel(), emb.astype(np.float32).ravel())


_IN_SPECS = [
    ("xln", (112, T), "float16"),
    ("w1a", (112, 64), "float16"),
    ("w1b", (112, 64), "float16"),
    ("w2a", (128, 64), "float16"),
    ("w2b", (128, 64), "float16"),
]


def _build_nc():
    from concourse import bacc, mybir

    nc = bacc.Bacc(
        "TRN2",
        target_bir_lowering=False,
        debug=False,
        enable_asserts=False,
        num_devices=8,
    )
    ins = {}
    for name, shape, dt in _IN_SPECS:
        h = nc.dram_tensor(name, list(shape), getattr(mybir.dt, dt),
                           kind="ExternalInput")
        ins[name] = h[:]
    out_h = nc.dram_tensor("out", [128, T // 4], mybir.dt.float32,
                           kind="ExternalOutput")
    build(nc, {"out": out_h[:]}, ins)
    nc.compile()
    return nc


def kernel(_trace=False, **inputs):
    from concourse.bass_utils import run_bass_kernel_spmd

    w1a, w1b, w2a, w2b, b2 = prep_weights(inputs)
    in_maps = []
    for g in range(G):
        in_maps.append(dict(xln=prep_x(g, inputs), w1a=w1a, w1b=w1b,
                            w2a=w2a, w2b=w2b))

    nc = _build_nc()
    res = run_bass_kernel_spmd(nc, in_maps, core_ids=list(range(G)),
                               trace=_trace)
    if _trace:
        print("HW exec time:", res.exec_time_ns, "ns  (mean:",
              res.mean_exec_time_ns, "ns, slowest core:",
              res.max_exec_time_core_id, ")")
        if res.instructions_and_trace:
            print("trace:", res.instructions_and_trace[1])

    attn = np.asarray(inputs["attn_bias"], np.float32)      # [G, 513, 513]
    virt = np.asarray(inputs["virt"], np.float32).reshape(H)
    outs = []
    for g, r in enumerate(res.results):
        # device rows: (g%4)*32 + h; cols: (g//4)*512 + c
        dev = r["out"].reshape(4, 32, NQ, 512).transpose(1, 2, 0, 3)
        dev = dev.reshape(32, T).reshape(32, N, N)          # [H, i, j]
        full = np.empty((H, NP1, NP1), np.float32)
        full[:, 1:, 1:] = dev + b2[:, None, None] + attn[g][None, 1:, 1:]
        full[:, 1:, 0] = attn[g][None, 1:, 0] + virt[:, None]
        full[:, 0, :] = attn[g][None, 0, :] + virt[:, None]
        edge_bias_host(g, inputs, full)
        outs.append(full)
    return np.stack(outs).astype(np.float32)
